# revision 66
# baseline (speedup 1.0000x reference)
"""Segment-mean reduction (grouped mean over sorted segment ids) on 8 trn2 cores.

Strategy (data-parallel over batch): each core handles one batch row.
out[g, :] = mean over rows s of feats with segment_ids[s] == g.

Host-side staging (inside kernel(), before upload):
  * Rows stay in their natural (sorted-by-segment-id) order: NO padding and NO
    gather. The 1024 groups split into 8 chunks of 128; a 128-row tile touches
    at most 2 adjacent chunks. The device schedule is shared across all 8
    cores: for every tile t it issues one matmul per chunk in the UNION (over
    cores) of chunks that tile t touches; cores where a (tile, chunk) pair is
    empty get an all(-1) local-id column -> all-zero one-hot -> adds exact 0.
  * feats are shipped as an fp16 hi/lo split (hi = fp16(x), lo = fp16(x - hi)),
    packed per tile as [128, 512] = [hi || lo]; adding the hi and lo halves of
    the 512-wide matmul output recovers ~fp32 accuracy with ONE matmul per
    (tile, chunk) pair.
  * DRAM layout is transposed to [128, T*512] so every DMA partition line is a
    large contiguous run (1 KiB per tile per partition).

Device program (static schedule):
  * sl (local-id columns, one per pair) rides the SP HWDGE ring ahead of the
    feats groups; rc rides the Activation ring; iota is generated on-chip.
  * Dummy matmuls on a gpsimd-zeroed tile warm the PE HAM clock gate (cold PE
    runs at 1.2 GHz, warm 2.4 GHz): 10 upfront during engine bringup, then
    one trailing each of the first 8 tiles and every other tile through 16,
    pacing the PE to the DMA ramp so it never idles long enough to
    re-throttle. They write psum[7], which chunk 7's first real matmul
    overwrites (start=True).
  * per pair (t, c): onehot[s, g] = (iota[g] == sl[s]) on DVE, then
    psum[c][:, 0:512] += onehot.T @ (hi || lo) on PE. Feats stream in 8-tile
    (1 MB) groups with a 4,2,2-tile tail: the final groups' completion
    semaphores (which lag their data by ~2 us) gate the last matmuls, so
    finer tail groups shrink that exposure.
  * per chunk c, emitted right after chunk c's last pair: st = psum_hi * rc
    on ACT, ot = psum_lo * rc + st on DVE, staged into [128, 512] pair tiles
    (chunks 0-5) or [128, 256] solo tiles (chunks 6, 7), then DMA'd on the
    Activation ring (never queued behind feats triggers on SP). Batching out
    DMAs keeps the ~8 round-robin HWDGE completion-semaphore lanes free of
    slow predecessors for the feats triggers; solo last chunks keep the final
    critical-path DMA small.

Per-chunk accumulation order (tiles ascending), the hi/lo split, and the
finish math match the reference baseline kernel bit-for-bit.

Per-core HBM traffic ~= feats (8.39 MB, no padding) + out (1 MB);
the stream runs at the ~430 GB/s SBUF-fabric ceiling => ~22 us stream.
"""

import numpy as np

import concourse.bass as bass
import concourse.bacc as bacc
import concourse.mybir as mybir
import concourse.tile as tile
from concourse.bass_utils import run_bass_kernel_spmd

F32 = mybir.dt.float32
F16 = mybir.dt.float16
P = 128  # partitions
H2 = 512  # hi || lo columns per tile
NDUMMY = 10   # upfront PE warmup matmuls (bridge engine bringup, ~4 us)
DUMMY_FULL = 8   # tiles with a trailing dummy matmul (pace PE ~= DMA ramp)
DUMMY_HALF = 16  # ...then every other tile, until this tile index


def _host_layout(seg_all: np.ndarray, G: int):
    """Shared (tile, chunk) pair schedule + per-core local-id columns."""
    R, S = seg_all.shape
    CH = G // P
    T = S // P
    assert T * P == S

    chunk_of = (seg_all // P).reshape(R, T, P)  # [R, T, 128]
    # union over cores of chunks touched by tile t (ascending)
    pairs = []  # list of (t, c)
    for t in range(T):
        cs = sorted(set(chunk_of[:, t, :].ravel().tolist()))
        for c in cs:
            pairs.append((t, c))
    NP_ = len(pairs)

    first = np.full(CH, -1, np.int64)
    last = np.full(CH, -1, np.int64)
    for j, (t, c) in enumerate(pairs):
        if first[c] < 0:
            first[c] = j
        last[c] = j

    # sl column per pair: local group id within chunk c, -1 if row not in c
    seg_t = seg_all.reshape(R, T, P)
    aux_sl = np.full((R, P, NP_), -1.0, np.float32)
    for j, (t, c) in enumerate(pairs):
        ids = seg_t[:, t, :]  # [R, 128]
        m = (ids // P) == c
        aux_sl[:, :, j] = np.where(m, (ids % P).astype(np.float32), -1.0)

    counts = np.stack(
        [np.bincount(seg_all[r], minlength=G) for r in range(R)]
    ).astype(np.float32)
    recip = (1.0 / np.maximum(counts, 1.0)).reshape(R, CH, P)
    aux_rc = np.ascontiguousarray(recip.transpose(0, 2, 1))

    return dict(T=T, CH=CH, NP=NP_, pairs=pairs, first=first, last=last,
                aux_sl=aux_sl, aux_rc=aux_rc)


def _dma_groups(T: int):
    """Skewed group sizes: small first (early first matmul) and small last
    (the final completion semaphores gate the last matmuls; finer groups
    shrink that exposure)."""
    tail = [4, 2, 1, 1]
    mid = T - sum(tail)
    assert mid >= 0 and mid % 8 == 0
    sizes = [8] * (mid // 8) + tail
    groups, t0 = [], 0
    for s in sizes:
        groups.append((t0, s))
        t0 += s
    return groups


def _build_program(H: int, G: int, lay):
    T, CH, NP_ = lay["T"], lay["CH"], lay["NP"]
    pairs = lay["pairs"]
    first, last = lay["first"], lay["last"]
    assert H2 == 2 * H

    # pairs grouped by tile for the emission loop
    pairs_of_tile = [[] for _ in range(T)]
    for j, (t, c) in enumerate(pairs):
        pairs_of_tile[t].append((j, c))

    nc = bacc.Bacc("TRN2", target_bir_lowering=False, debug=False, num_devices=8)
    hl_d = nc.dram_tensor("feats_hl", [P, T * H2], F16, kind="ExternalInput")
    sl_d = nc.dram_tensor("aux_sl", [P, NP_], F32, kind="ExternalInput")
    rc_d = nc.dram_tensor("aux_rc", [P, CH], F32, kind="ExternalInput")
    # transposed output: col c*H + h, partition p <=> out[c*128 + p, h]
    out_d = nc.dram_tensor("out", [P, CH * H], F32, kind="ExternalOutput")

    with tile.TileContext(nc) as tc:
        with (
            tc.tile_pool(name="const", bufs=1) as constp,
            tc.tile_pool(name="feats", bufs=8) as fpool,
            tc.tile_pool(name="mt", bufs=12) as mtpool,
            tc.tile_pool(name="stp", bufs=2) as stpool,
            tc.tile_pool(name="outp", bufs=2) as opool,
            tc.tile_pool(name="psum", bufs=1, space="PSUM") as pp,
        ):
            # the SP ring carries ONLY feats (its first instruction is the
            # first feats trigger -> the stream starts at engine bringup);
            # sl and rc ride the Activation ring, which comes up at the same
            # time and is otherwise idle until the first chunk finish
            sl_t = constp.tile([P, NP_], F32, tag="sl")
            nc.scalar.dma_start(sl_t[:], sl_d.ap())
            rc_t = constp.tile([P, CH], F32, tag="rc")
            nc.scalar.dma_start(rc_t[:], rc_d.ap())
            # dummy memset first on gpsimd so PE warmup starts earliest
            dummy = constp.tile([P, H2], F16, tag="dummy")
            nc.gpsimd.memset(dummy[:], 0.0)
            iota_t = constp.tile([P, P], F16, tag="iota")
            nc.gpsimd.iota(
                iota_t[:], [[1, P]], base=0, channel_multiplier=0,
                allow_small_or_imprecise_dtypes=True,
            )

            psum_tiles = [
                pp.tile([P, H2], F32, tag=f"ps{c}", name=f"ps{c}") for c in range(CH)
            ]

            # PE HAM warmup on the gpsimd-zeroed tile; psum[CH-1] is
            # overwritten by its first real matmul (start=True).
            def dummy_mm():
                nc.tensor.matmul(
                    psum_tiles[CH - 1][:], dummy[:, :P], dummy[:],
                    start=True, stop=True,
                )

            for _ in range(NDUMMY):
                dummy_mm()

            # chunks 0-5 pair up in [P, 2H] tiles, one out DMA per pair
            # (fewer DMAs keeps the round-robin completion-sem lanes free of
            # slow out-DMA predecessors for the feats triggers); chunks 6, 7
            # ship solo so the final critical-path DMA is as small as possible
            otp_box = [None]

            def finish(c):
                paired = c < CH - 2
                if not paired or c % 2 == 0:
                    w = 2 * H if paired else H
                    otp_box[0] = opool.tile([P, w], F32, tag="ot", name="ot")
                otp = otp_box[0]
                lo = (c % 2) * H if paired else 0
                sub = otp[:, lo:lo + H]
                if first[c] >= 0:
                    st = stpool.tile([P, H], F32, tag="st", name="st")
                    # st = psum_hi * (1/count) on ACT (single PSUM operand)
                    nc.scalar.activation(
                        st[:], psum_tiles[c][:, :H],
                        mybir.ActivationFunctionType.Copy,
                        scale=rc_t[:, c:c + 1],
                    )
                    # ot = (psum_lo * (1/count)) + st on DVE
                    nc.vector.scalar_tensor_tensor(
                        sub, psum_tiles[c][:, H:], rc_t[:, c:c + 1], st[:],
                        mybir.AluOpType.mult, mybir.AluOpType.add,
                    )
                else:
                    nc.vector.memset(sub, 0.0)
                if not paired:
                    # solo triggers ride the SP ring (idle after the feats
                    # triggers) so they never sit between ACT c and ACT c+1
                    # in the Activation queue on the exit critical path
                    nc.sync.dma_start(
                        out_d.ap()[:, c * H:(c + 1) * H], otp[:])
                elif c % 2 == 1:
                    # out DMA on the Activation ring for chunks c-1, c
                    nc.scalar.dma_start(
                        out_d.ap()[:, (c - 1) * H:(c + 1) * H], otp[:])

            for t0, nt in _dma_groups(T):
                ft = fpool.tile([P, 8 * H2], F16, tag="ft")
                nc.sync.dma_start(
                    ft[:, :nt * H2], hl_d.ap()[:, t0 * H2:(t0 + nt) * H2])
                for tt in range(nt):
                    t = t0 + tt
                    for j, c in pairs_of_tile[t]:
                        mt = mtpool.tile([P, P], F16, tag="mt", name="mt")
                        # onehot[s, g] = (iota[g] == sl[s]) on DVE
                        nc.vector.tensor_scalar(
                            mt[:],
                            iota_t[:],
                            sl_t[:, j:j + 1],
                            None,
                            mybir.AluOpType.is_equal,
                        )
                        nc.tensor.matmul(
                            psum_tiles[c][:], mt[:],
                            ft[:, tt * H2:(tt + 1) * H2],
                            start=(j == first[c]), stop=(j == last[c]),
                        )
                        if j == last[c]:
                            finish(c)
                    # keep the PE busy through the DMA ramp so HAM never
                    # re-throttles: trailing dummies paced to the early
                    # delivery rate
                    if t < DUMMY_FULL or (t < DUMMY_HALF and t % 2 == 0):
                        dummy_mm()
            for c in range(CH):
                if first[c] < 0:
                    finish(c)

    nc.compile()
    return nc


def kernel(feats, segment_ids, num_groups, _trace=False):
    feats = np.ascontiguousarray(np.asarray(feats, dtype=np.float32))
    seg_all = np.ascontiguousarray(np.asarray(segment_ids, dtype=np.int32))
    G = int(num_groups)
    B, S, H = feats.shape
    assert seg_all.shape == (B, S) and B == 8 and G % P == 0

    lay = _host_layout(seg_all, G)
    T = lay["T"]
    nc = _build_program(H, G, lay)

    in_maps = []
    for r in range(B):
        fr = feats[r]  # [S, H] fp32, rows already sorted by segment id
        hi = fr.astype(np.float16)
        lo = (fr - hi.astype(np.float32)).astype(np.float16)
        hl = np.concatenate([hi, lo], axis=1)  # [S, 2H]
        # transpose to [128, T*2H]: partition p, col t*2H+h <- row t*128+p
        hl_t = np.ascontiguousarray(
            hl.reshape(T, P, H2).transpose(1, 0, 2).reshape(P, T * H2))
        in_maps.append({
            "feats_hl": hl_t,
            "aux_sl": lay["aux_sl"][r],
            "aux_rc": lay["aux_rc"][r],
        })
    res = run_bass_kernel_spmd(nc, in_maps, list(range(B)), trace=_trace)
    CH = lay["CH"]
    out = np.stack([
        # [128, CH*H] -> [G, H]: out[c*128+p, h] = res[p, c*H+h]
        np.ascontiguousarray(
            res.results[r]["out"].reshape(P, CH, H).transpose(1, 0, 2)
        ).reshape(G, H)
        for r in range(B)
    ])
    if _trace:
        return out, res
    return out


# revision 69
# speedup vs baseline: 1.0026x; 1.0026x over previous
"""Segment-mean reduction (grouped mean over sorted segment ids) on 8 trn2 cores.

Strategy (data-parallel over batch): each core handles one batch row.
out[g, :] = mean over rows s of feats with segment_ids[s] == g.

Host-side staging (inside kernel(), before upload):
  * Rows stay in their natural (sorted-by-segment-id) order: NO padding and NO
    gather. The 1024 groups split into 8 chunks of 128; a 128-row tile touches
    at most 2 adjacent chunks. The device schedule is shared across all 8
    cores: for every tile t it issues one matmul per chunk in the UNION (over
    cores) of chunks that tile t touches; cores where a (tile, chunk) pair is
    empty get an all(-1) local-id column -> all-zero one-hot -> adds exact 0.
  * feats are shipped as an fp16 hi/lo split (hi = fp16(x), lo = fp16(x - hi)),
    packed per tile as [128, 512] = [hi || lo]; adding the hi and lo halves of
    the 512-wide matmul output recovers ~fp32 accuracy with ONE matmul per
    (tile, chunk) pair.
  * DRAM layout is transposed to [128, T*512] so every DMA partition line is a
    large contiguous run (1 KiB per tile per partition).

Device program (static schedule):
  * sl (local-id columns, one per pair) rides the SP HWDGE ring ahead of the
    feats groups; rc rides the Activation ring; iota is generated on-chip.
  * Dummy matmuls on a gpsimd-zeroed tile warm the PE HAM clock gate (cold PE
    runs at 1.2 GHz, warm 2.4 GHz): 10 upfront during engine bringup, then
    one trailing each of the first 8 tiles and every other tile through 16,
    pacing the PE to the DMA ramp so it never idles long enough to
    re-throttle. They write psum[7], which chunk 7's first real matmul
    overwrites (start=True).
  * per pair (t, c): onehot[s, g] = (iota[g] == sl[s]) on DVE, then
    psum[c][:, 0:512] += onehot.T @ (hi || lo) on PE. Feats stream in 8-tile
    (1 MB) groups with a 4,2,2-tile tail: the final groups' completion
    semaphores (which lag their data by ~2 us) gate the last matmuls, so
    finer tail groups shrink that exposure.
  * per chunk c, emitted right after chunk c's last pair: st = psum_hi * rc
    on ACT, ot = psum_lo * rc + st on DVE, staged into [128, 512] pair tiles
    (chunks 0-5) or [128, 256] solo tiles (chunks 6, 7), then DMA'd on the
    Activation ring (never queued behind feats triggers on SP). Batching out
    DMAs keeps the ~8 round-robin HWDGE completion-semaphore lanes free of
    slow predecessors for the feats triggers; solo last chunks keep the final
    critical-path DMA small.

Per-chunk accumulation order (tiles ascending), the hi/lo split, and the
finish math match the reference baseline kernel bit-for-bit.

Per-core HBM traffic ~= feats (8.39 MB, no padding) + out (1 MB);
the stream runs at the ~430 GB/s SBUF-fabric ceiling => ~22 us stream.
"""

import numpy as np

import concourse.bass as bass
import concourse.bacc as bacc
import concourse.mybir as mybir
import concourse.tile as tile
from concourse.bass_utils import run_bass_kernel_spmd

F32 = mybir.dt.float32
F16 = mybir.dt.float16
P = 128  # partitions
H2 = 512  # hi || lo columns per tile
NDUMMY = 10   # upfront PE warmup matmuls (bridge engine bringup, ~4 us)
DUMMY_FULL = 8   # tiles with a trailing dummy matmul (pace PE ~= DMA ramp)
DUMMY_HALF = 16  # ...then every other tile, until this tile index


def _host_layout(seg_all: np.ndarray, G: int):
    """Shared (tile, chunk) pair schedule + per-core local-id columns."""
    R, S = seg_all.shape
    CH = G // P
    T = S // P
    assert T * P == S

    chunk_of = (seg_all // P).reshape(R, T, P)  # [R, T, 128]
    # union over cores of chunks touched by tile t (ascending)
    pairs = []  # list of (t, c)
    for t in range(T):
        cs = sorted(set(chunk_of[:, t, :].ravel().tolist()))
        for c in cs:
            pairs.append((t, c))
    NP_ = len(pairs)

    first = np.full(CH, -1, np.int64)
    last = np.full(CH, -1, np.int64)
    for j, (t, c) in enumerate(pairs):
        if first[c] < 0:
            first[c] = j
        last[c] = j

    # sl column per pair: local group id within chunk c, -1 if row not in c
    seg_t = seg_all.reshape(R, T, P)
    aux_sl = np.full((R, P, NP_), -1.0, np.float32)
    for j, (t, c) in enumerate(pairs):
        ids = seg_t[:, t, :]  # [R, 128]
        m = (ids // P) == c
        aux_sl[:, :, j] = np.where(m, (ids % P).astype(np.float32), -1.0)

    counts = np.stack(
        [np.bincount(seg_all[r], minlength=G) for r in range(R)]
    ).astype(np.float32)
    recip = (1.0 / np.maximum(counts, 1.0)).reshape(R, CH, P)
    aux_rc = np.ascontiguousarray(recip.transpose(0, 2, 1))

    return dict(T=T, CH=CH, NP=NP_, pairs=pairs, first=first, last=last,
                aux_sl=aux_sl, aux_rc=aux_rc)


def _dma_groups(T: int):
    """Skewed group sizes: small first (early first matmul) and small last
    (the final completion semaphores gate the last matmuls; finer groups
    shrink that exposure)."""
    tail = [4, 2, 1, 1]
    mid = T - sum(tail)
    assert mid >= 0 and mid % 8 == 0
    sizes = [8] * (mid // 8) + tail
    groups, t0 = [], 0
    for s in sizes:
        groups.append((t0, s))
        t0 += s
    return groups


def _build_program(H: int, G: int, lay):
    T, CH, NP_ = lay["T"], lay["CH"], lay["NP"]
    pairs = lay["pairs"]
    first, last = lay["first"], lay["last"]
    assert H2 == 2 * H

    # pairs grouped by tile for the emission loop
    pairs_of_tile = [[] for _ in range(T)]
    for j, (t, c) in enumerate(pairs):
        pairs_of_tile[t].append((j, c))

    nc = bacc.Bacc("TRN2", target_bir_lowering=False, debug=False, num_devices=8)
    hl_d = nc.dram_tensor("feats_hl", [P, T * H2], F16, kind="ExternalInput")
    sl_d = nc.dram_tensor("aux_sl", [P, NP_], F32, kind="ExternalInput")
    rc_d = nc.dram_tensor("aux_rc", [P, CH], F32, kind="ExternalInput")
    # transposed output: col c*H + h, partition p <=> out[c*128 + p, h]
    out_d = nc.dram_tensor("out", [P, CH * H], F32, kind="ExternalOutput")

    with tile.TileContext(nc) as tc:
        with (
            tc.tile_pool(name="const", bufs=1) as constp,
            tc.tile_pool(name="feats", bufs=8) as fpool,
            tc.tile_pool(name="mt", bufs=12) as mtpool,
            tc.tile_pool(name="stp", bufs=4) as stpool,
            tc.tile_pool(name="outp", bufs=2) as opool,
            tc.tile_pool(name="psum", bufs=1, space="PSUM") as pp,
        ):
            # the SP ring carries ONLY feats (its first instruction is the
            # first feats trigger -> the stream starts at engine bringup);
            # sl and rc ride the Activation ring, which comes up at the same
            # time and is otherwise idle until the first chunk finish
            sl_t = constp.tile([P, NP_], F32, tag="sl")
            nc.scalar.dma_start(sl_t[:], sl_d.ap())
            rc_t = constp.tile([P, CH], F32, tag="rc")
            nc.scalar.dma_start(rc_t[:], rc_d.ap())
            # dummy memset first on gpsimd so PE warmup starts earliest
            dummy = constp.tile([P, H2], F16, tag="dummy")
            nc.gpsimd.memset(dummy[:], 0.0)
            iota_t = constp.tile([P, P], F16, tag="iota")
            nc.gpsimd.iota(
                iota_t[:], [[1, P]], base=0, channel_multiplier=0,
                allow_small_or_imprecise_dtypes=True,
            )

            psum_tiles = [
                pp.tile([P, H2], F32, tag=f"ps{c}", name=f"ps{c}") for c in range(CH)
            ]

            # PE HAM warmup on the gpsimd-zeroed tile; psum[CH-1] is
            # overwritten by its first real matmul (start=True).
            def dummy_mm():
                nc.tensor.matmul(
                    psum_tiles[CH - 1][:], dummy[:, :P], dummy[:],
                    start=True, stop=True,
                )

            for _ in range(NDUMMY):
                dummy_mm()

            # chunks 0-5 pair up in [P, 2H] tiles, one out DMA per pair
            # (fewer DMAs keeps the round-robin completion-sem lanes free of
            # slow out-DMA predecessors for the feats triggers); chunks 6, 7
            # ship solo so the final critical-path DMA is as small as possible
            otp_box = [None]
            deferred = []  # (c, otp, sub, st) with STT + trigger pending

            def emit_stt(c, sub, st):
                # ot = (psum_lo * (1/count)) + st on DVE
                nc.vector.scalar_tensor_tensor(
                    sub, psum_tiles[c][:, H:], rc_t[:, c:c + 1], st[:],
                    mybir.AluOpType.mult, mybir.AluOpType.add,
                )

            def emit_trigger(c, otp, on_sync):
                paired = c < CH - 2
                eng = nc.sync if on_sync else nc.scalar
                if not paired:
                    eng.dma_start(out_d.ap()[:, c * H:(c + 1) * H], otp[:])
                elif c % 2 == 1:
                    eng.dma_start(
                        out_d.ap()[:, (c - 1) * H:(c + 1) * H], otp[:])

            def finish(c):
                paired = c < CH - 2
                if not paired or c % 2 == 0:
                    w = 2 * H if paired else H
                    otp_box[0] = opool.tile([P, w], F32, tag="ot", name="ot")
                otp = otp_box[0]
                lo = (c % 2) * H if paired else 0
                sub = otp[:, lo:lo + H]
                st = None
                if first[c] >= 0:
                    st = stpool.tile([P, H], F32, tag="st", name="st")
                    # st = psum_hi * (1/count) on ACT (single PSUM operand)
                    nc.scalar.activation(
                        st[:], psum_tiles[c][:, :H],
                        mybir.ActivationFunctionType.Copy,
                        scale=rc_t[:, c:c + 1],
                    )
                    # The DVE runs in program order: an STT emitted mid-loop
                    # waits on its psum and BLOCKS the one-hot builds behind
                    # it. For the last chunks that wait lands on the exit
                    # critical path, so defer their STT + trigger past all
                    # one-hot emission; deferred triggers ride the (then
                    # idle) SP ring.
                    if CH - 3 <= c < CH - 1:
                        deferred.append((c, otp, sub, st))
                        return
                else:
                    nc.vector.memset(sub, 0.0)
                if c == CH - 1:
                    for dc, dotp, dsub, dst in deferred:
                        emit_stt(dc, dsub, dst)
                    if st is not None:
                        emit_stt(c, sub, st)
                    for dc, dotp, dsub, dst in deferred:
                        emit_trigger(dc, dotp, on_sync=True)
                    emit_trigger(c, otp, on_sync=True)
                else:
                    if st is not None:
                        emit_stt(c, sub, st)
                    emit_trigger(c, otp, on_sync=False)

            for t0, nt in _dma_groups(T):
                ft = fpool.tile([P, 8 * H2], F16, tag="ft")
                nc.sync.dma_start(
                    ft[:, :nt * H2], hl_d.ap()[:, t0 * H2:(t0 + nt) * H2])
                for tt in range(nt):
                    t = t0 + tt
                    for j, c in pairs_of_tile[t]:
                        mt = mtpool.tile([P, P], F16, tag="mt", name="mt")
                        # onehot[s, g] = (iota[g] == sl[s]) on DVE
                        nc.vector.tensor_scalar(
                            mt[:],
                            iota_t[:],
                            sl_t[:, j:j + 1],
                            None,
                            mybir.AluOpType.is_equal,
                        )
                        nc.tensor.matmul(
                            psum_tiles[c][:], mt[:],
                            ft[:, tt * H2:(tt + 1) * H2],
                            start=(j == first[c]), stop=(j == last[c]),
                        )
                        if j == last[c]:
                            finish(c)
                    # keep the PE busy through the DMA ramp so HAM never
                    # re-throttles: trailing dummies paced to the early
                    # delivery rate
                    if t < DUMMY_FULL or (t < DUMMY_HALF and t % 2 == 0):
                        dummy_mm()
            for c in range(CH):
                if first[c] < 0:
                    finish(c)

    nc.compile()
    return nc


def kernel(feats, segment_ids, num_groups, _trace=False):
    feats = np.ascontiguousarray(np.asarray(feats, dtype=np.float32))
    seg_all = np.ascontiguousarray(np.asarray(segment_ids, dtype=np.int32))
    G = int(num_groups)
    B, S, H = feats.shape
    assert seg_all.shape == (B, S) and B == 8 and G % P == 0

    lay = _host_layout(seg_all, G)
    T = lay["T"]
    nc = _build_program(H, G, lay)

    in_maps = []
    for r in range(B):
        fr = feats[r]  # [S, H] fp32, rows already sorted by segment id
        hi = fr.astype(np.float16)
        lo = (fr - hi.astype(np.float32)).astype(np.float16)
        hl = np.concatenate([hi, lo], axis=1)  # [S, 2H]
        # transpose to [128, T*2H]: partition p, col t*2H+h <- row t*128+p
        hl_t = np.ascontiguousarray(
            hl.reshape(T, P, H2).transpose(1, 0, 2).reshape(P, T * H2))
        in_maps.append({
            "feats_hl": hl_t,
            "aux_sl": lay["aux_sl"][r],
            "aux_rc": lay["aux_rc"][r],
        })
    res = run_bass_kernel_spmd(nc, in_maps, list(range(B)), trace=_trace)
    CH = lay["CH"]
    out = np.stack([
        # [128, CH*H] -> [G, H]: out[c*128+p, h] = res[p, c*H+h]
        np.ascontiguousarray(
            res.results[r]["out"].reshape(P, CH, H).transpose(1, 0, 2)
        ).reshape(G, H)
        for r in range(B)
    ])
    if _trace:
        return out, res
    return out


# revision 71
# speedup vs baseline: 1.0976x; 1.0948x over previous
"""Segment-mean reduction (grouped mean over sorted segment ids) on 8 trn2 cores.

Strategy (data-parallel over batch): each core handles one batch row.
out[g, :] = mean over rows s of feats with segment_ids[s] == g.

Host-side staging (inside kernel(), before upload):
  * Rows stay in their natural (sorted-by-segment-id) order: NO padding and NO
    gather. The 1024 groups split into 8 chunks of 128; a 128-row tile touches
    at most 2 adjacent chunks. The device schedule is shared across all 8
    cores: for every tile t it issues one matmul per chunk in the UNION (over
    cores) of chunks that tile t touches; cores where a (tile, chunk) pair is
    empty get an all(-1) local-id column -> all-zero one-hot -> adds exact 0.
  * feats are shipped as an fp16 hi/lo split (hi = fp16(x), lo = fp16(x - hi)),
    packed per tile as [128, 512] = [hi || lo]; adding the hi and lo halves of
    the 512-wide matmul output recovers ~fp32 accuracy with ONE matmul per
    (tile, chunk) pair.
  * DRAM layout is transposed to [128, T*512] so every DMA partition line is a
    large contiguous run (1 KiB per tile per partition).

Device program (static schedule):
  * sl (local-id columns, one per pair) rides the SP HWDGE ring ahead of the
    feats groups; rc rides the Activation ring; iota is generated on-chip.
  * Dummy matmuls on a gpsimd-zeroed tile warm the PE HAM clock gate (cold PE
    runs at 1.2 GHz, warm 2.4 GHz): 10 upfront during engine bringup, then
    one trailing each of the first 8 tiles and every other tile through 16,
    pacing the PE to the DMA ramp so it never idles long enough to
    re-throttle. They write psum[7], which chunk 7's first real matmul
    overwrites (start=True).
  * per pair (t, c): onehot[s, g] = (iota[g] == sl[s]) on DVE, then
    psum[c][:, 0:512] += onehot.T @ (hi || lo) on PE. Feats stream in 8-tile
    (1 MB) groups with a 4,2,2-tile tail: the final groups' completion
    semaphores (which lag their data by ~2 us) gate the last matmuls, so
    finer tail groups shrink that exposure.
  * per chunk c, emitted right after chunk c's last pair: st = psum_hi * rc
    on ACT, ot = psum_lo * rc + st on DVE, staged into [128, 512] pair tiles
    (chunks 0-5) or [128, 256] solo tiles (chunks 6, 7), then DMA'd on the
    Activation ring (never queued behind feats triggers on SP). Batching out
    DMAs keeps the ~8 round-robin HWDGE completion-semaphore lanes free of
    slow predecessors for the feats triggers; solo last chunks keep the final
    critical-path DMA small.

Per-chunk accumulation order (tiles ascending), the hi/lo split, and the
finish math match the reference baseline kernel bit-for-bit.

Per-core HBM traffic ~= feats (8.39 MB, no padding) + out (1 MB);
the stream runs at the ~430 GB/s SBUF-fabric ceiling => ~22 us stream.
"""

import numpy as np

import concourse.bass as bass
import concourse.bacc as bacc
import concourse.mybir as mybir
import concourse.tile as tile
from concourse.bass_utils import run_bass_kernel_spmd

F32 = mybir.dt.float32
F16 = mybir.dt.float16
P = 128  # partitions
H2 = 512  # hi || lo columns per tile
NDUMMY = 10   # upfront PE warmup matmuls (bridge engine bringup, ~4 us)
DUMMY_FULL = 8   # tiles with a trailing dummy matmul (pace PE ~= DMA ramp)
DUMMY_HALF = 16  # ...then every other tile, until this tile index


def _host_layout(seg_all: np.ndarray, G: int):
    """Shared (tile, chunk) pair schedule + per-core local-id columns."""
    R, S = seg_all.shape
    CH = G // P
    T = S // P
    assert T * P == S

    chunk_of = (seg_all // P).reshape(R, T, P)  # [R, T, 128]
    # union over cores of chunks touched by tile t (ascending)
    pairs = []  # list of (t, c)
    for t in range(T):
        cs = sorted(set(chunk_of[:, t, :].ravel().tolist()))
        for c in cs:
            pairs.append((t, c))
    NP_ = len(pairs)

    first = np.full(CH, -1, np.int64)
    last = np.full(CH, -1, np.int64)
    for j, (t, c) in enumerate(pairs):
        if first[c] < 0:
            first[c] = j
        last[c] = j

    # sl column per pair: local group id within chunk c, -1 if row not in c
    seg_t = seg_all.reshape(R, T, P)
    aux_sl = np.full((R, P, NP_), -1.0, np.float32)
    for j, (t, c) in enumerate(pairs):
        ids = seg_t[:, t, :]  # [R, 128]
        m = (ids // P) == c
        aux_sl[:, :, j] = np.where(m, (ids % P).astype(np.float32), -1.0)

    counts = np.stack(
        [np.bincount(seg_all[r], minlength=G) for r in range(R)]
    ).astype(np.float32)
    recip = (1.0 / np.maximum(counts, 1.0)).reshape(R, CH, P)
    aux_rc = np.ascontiguousarray(recip.transpose(0, 2, 1))

    return dict(T=T, CH=CH, NP=NP_, pairs=pairs, first=first, last=last,
                aux_sl=aux_sl, aux_rc=aux_rc)


def _dma_groups(T: int):
    """Skewed group sizes: small first (early first matmul) and small last
    (the final completion semaphores gate the last matmuls; finer groups
    shrink that exposure)."""
    tail = [4, 2, 1, 1]
    mid = T - sum(tail)
    assert mid >= 0 and mid % 8 == 0
    sizes = [8] * (mid // 8) + tail
    groups, t0 = [], 0
    for s in sizes:
        groups.append((t0, s))
        t0 += s
    return groups


def _build_program(H: int, G: int, lay):
    T, CH, NP_ = lay["T"], lay["CH"], lay["NP"]
    pairs = lay["pairs"]
    first, last = lay["first"], lay["last"]
    assert H2 == 2 * H

    # pairs grouped by tile for the emission loop
    pairs_of_tile = [[] for _ in range(T)]
    for j, (t, c) in enumerate(pairs):
        pairs_of_tile[t].append((j, c))

    nc = bacc.Bacc("TRN2", target_bir_lowering=False, debug=False, num_devices=8)
    hl_d = nc.dram_tensor("feats_hl", [P, T * H2], F16, kind="ExternalInput")
    sl_d = nc.dram_tensor("aux_sl", [P, NP_], F32, kind="ExternalInput")
    rc_d = nc.dram_tensor("aux_rc", [P, CH], F32, kind="ExternalInput")
    # transposed output: col c*H + h, partition p <=> out[c*128 + p, h]
    out_d = nc.dram_tensor("out", [P, CH * H], F32, kind="ExternalOutput")

    with tile.TileContext(nc) as tc:
        with (
            tc.tile_pool(name="const", bufs=1) as constp,
            tc.tile_pool(name="feats", bufs=8) as fpool,
            tc.tile_pool(name="mt", bufs=12) as mtpool,
            tc.tile_pool(name="stp", bufs=4) as stpool,
            tc.tile_pool(name="outp", bufs=2) as opool,
            tc.tile_pool(name="psum", bufs=1, space="PSUM") as pp,
        ):
            # the SP ring carries ONLY feats (its first instruction is the
            # first feats trigger -> the stream starts at engine bringup);
            # sl and rc ride the Activation ring, which comes up at the same
            # time and is otherwise idle until the first chunk finish
            sl_t = constp.tile([P, NP_], F32, tag="sl")
            nc.scalar.dma_start(sl_t[:], sl_d.ap())
            rc_t = constp.tile([P, CH], F32, tag="rc")
            nc.scalar.dma_start(rc_t[:], rc_d.ap())
            # dummy memset first on gpsimd so PE warmup starts earliest
            dummy = constp.tile([P, H2], F16, tag="dummy")
            nc.gpsimd.memset(dummy[:], 0.0)
            iota_t = constp.tile([P, P], F16, tag="iota")
            nc.gpsimd.iota(
                iota_t[:], [[1, P]], base=0, channel_multiplier=0,
                allow_small_or_imprecise_dtypes=True,
            )

            psum_tiles = [
                pp.tile([P, H2], F32, tag=f"ps{c}", name=f"ps{c}") for c in range(CH)
            ]

            # PE HAM warmup on the gpsimd-zeroed tile; psum[CH-1] is
            # overwritten by its first real matmul (start=True).
            def dummy_mm():
                nc.tensor.matmul(
                    psum_tiles[CH - 1][:], dummy[:, :P], dummy[:],
                    start=True, stop=True,
                )

            for _ in range(NDUMMY):
                dummy_mm()

            # chunks 0-5 pair up in [P, 2H] tiles, one out DMA per pair
            # (fewer DMAs keeps the round-robin completion-sem lanes free of
            # slow out-DMA predecessors for the feats triggers); chunks 6, 7
            # ship solo so the final critical-path DMA is as small as possible
            otp_box = [None]
            deferred = []  # (c, otp, sub, st) with STT + trigger pending

            def emit_stt(c, sub, st):
                # ot = (psum_lo * (1/count)) + st on DVE
                nc.vector.scalar_tensor_tensor(
                    sub, psum_tiles[c][:, H:], rc_t[:, c:c + 1], st[:],
                    mybir.AluOpType.mult, mybir.AluOpType.add,
                )

            def emit_trigger(c, otp, on_sync):
                paired = c < CH - 2
                eng = nc.sync if on_sync else nc.scalar
                if not paired:
                    eng.dma_start(out_d.ap()[:, c * H:(c + 1) * H], otp[:])
                elif c % 2 == 1:
                    eng.dma_start(
                        out_d.ap()[:, (c - 1) * H:(c + 1) * H], otp[:])

            def finish(c):
                paired = c < CH - 2
                if not paired or c % 2 == 0:
                    w = 2 * H if paired else H
                    otp_box[0] = opool.tile([P, w], F32, tag="ot", name="ot")
                otp = otp_box[0]
                lo = (c % 2) * H if paired else 0
                sub = otp[:, lo:lo + H]
                st = None
                if first[c] >= 0:
                    st = stpool.tile([P, H], F32, tag="st", name="st")
                    if c < CH - 1:
                        # st = psum_hi * (1/count) on ACT
                        nc.scalar.activation(
                            st[:], psum_tiles[c][:, :H],
                            mybir.ActivationFunctionType.Copy,
                            scale=rc_t[:, c:c + 1],
                        )
                    # The DVE runs in program order: an STT emitted mid-loop
                    # waits on its psum and BLOCKS the one-hot builds behind
                    # it. For the last chunks that wait lands on the exit
                    # critical path, so defer their STT + trigger past all
                    # one-hot emission; deferred triggers ride the (then
                    # idle) SP ring.
                    if CH - 3 <= c < CH - 1:
                        deferred.append((c, otp, sub, st))
                        return
                else:
                    nc.vector.memset(sub, 0.0)
                if c == CH - 1:
                    for dc, dotp, dsub, dst in deferred:
                        emit_stt(dc, dsub, dst)
                    if st is not None:
                        # last chunk: hi-scale on DVE too (bit-identical fp32
                        # multiply) — keeps the whole exit chain on one
                        # in-order engine, skipping the ~1 us ACT->DVE
                        # semaphore handoff
                        nc.vector.tensor_scalar(
                            st[:], psum_tiles[c][:, :H], rc_t[:, c:c + 1],
                            None, mybir.AluOpType.mult,
                        )
                        emit_stt(c, sub, st)
                    for dc, dotp, dsub, dst in deferred:
                        emit_trigger(dc, dotp, on_sync=True)
                    emit_trigger(c, otp, on_sync=True)
                else:
                    if st is not None:
                        emit_stt(c, sub, st)
                    emit_trigger(c, otp, on_sync=False)

            for t0, nt in _dma_groups(T):
                ft = fpool.tile([P, 8 * H2], F16, tag="ft")
                nc.sync.dma_start(
                    ft[:, :nt * H2], hl_d.ap()[:, t0 * H2:(t0 + nt) * H2])
                for tt in range(nt):
                    t = t0 + tt
                    for j, c in pairs_of_tile[t]:
                        mt = mtpool.tile([P, P], F16, tag="mt", name="mt")
                        # onehot[s, g] = (iota[g] == sl[s]) on DVE
                        nc.vector.tensor_scalar(
                            mt[:],
                            iota_t[:],
                            sl_t[:, j:j + 1],
                            None,
                            mybir.AluOpType.is_equal,
                        )
                        nc.tensor.matmul(
                            psum_tiles[c][:], mt[:],
                            ft[:, tt * H2:(tt + 1) * H2],
                            start=(j == first[c]), stop=(j == last[c]),
                        )
                        if j == last[c]:
                            finish(c)
                    # keep the PE busy through the DMA ramp so HAM never
                    # re-throttles: trailing dummies paced to the early
                    # delivery rate
                    if t < DUMMY_FULL or (t < DUMMY_HALF and t % 2 == 0):
                        dummy_mm()
            for c in range(CH):
                if first[c] < 0:
                    finish(c)

    nc.compile()
    return nc


def kernel(feats, segment_ids, num_groups, _trace=False):
    feats = np.ascontiguousarray(np.asarray(feats, dtype=np.float32))
    seg_all = np.ascontiguousarray(np.asarray(segment_ids, dtype=np.int32))
    G = int(num_groups)
    B, S, H = feats.shape
    assert seg_all.shape == (B, S) and B == 8 and G % P == 0

    lay = _host_layout(seg_all, G)
    T = lay["T"]
    nc = _build_program(H, G, lay)

    in_maps = []
    for r in range(B):
        fr = feats[r]  # [S, H] fp32, rows already sorted by segment id
        hi = fr.astype(np.float16)
        lo = (fr - hi.astype(np.float32)).astype(np.float16)
        hl = np.concatenate([hi, lo], axis=1)  # [S, 2H]
        # transpose to [128, T*2H]: partition p, col t*2H+h <- row t*128+p
        hl_t = np.ascontiguousarray(
            hl.reshape(T, P, H2).transpose(1, 0, 2).reshape(P, T * H2))
        in_maps.append({
            "feats_hl": hl_t,
            "aux_sl": lay["aux_sl"][r],
            "aux_rc": lay["aux_rc"][r],
        })
    res = run_bass_kernel_spmd(nc, in_maps, list(range(B)), trace=_trace)
    CH = lay["CH"]
    out = np.stack([
        # [128, CH*H] -> [G, H]: out[c*128+p, h] = res[p, c*H+h]
        np.ascontiguousarray(
            res.results[r]["out"].reshape(P, CH, H).transpose(1, 0, 2)
        ).reshape(G, H)
        for r in range(B)
    ])
    if _trace:
        return out, res
    return out
